# revision 1
# baseline (speedup 1.0000x reference)
"""DeepSTN on 8 Trainium2 NeuronCores.

v2 distribution: every core computes the (cheap) conv/pointwise path for the
FULL batch (replicated); the huge plus_conv GEMM is output-channel-sharded
(each core: full batch x its 512 of 4096 outputs, weights bf16 pre-chunked on
host). Per ResPlus iteration only ONE collective: AllGather of the GEMM
output shards. The GEMM consumes locally-transposed z2, so weight streaming
and the GEMM never wait on a collective. All BN affines folded on host.
"""
import numpy as np
import ml_dtypes

B, H, W = 32, 32, 16
HW = H * W            # 512
NC = 8                # cores
NFF = B * HW          # 16384 free elems per channel (full batch)
C = 64                # cpt channels
RP = 4                # ResPlus iterations (reference semantics)
KCH = 256             # GEMM k-chunks of 128 (64ch*512hw / 128)
OSH = 512             # output shard (4096 / 8)
EPS = 1e-5
PADR, PADC = H + 2, W + 2   # 34, 18
PB = PADR * PADC            # 612 per sample

_HANDLE = {}

import os as _os
DEFAULT_CFG = (int(_os.environ.get("KRP", "4")),
               _os.environ.get("KCOLL", "1") == "1",
               _os.environ.get("KGEMM", "1") == "1",
               _os.environ.get("KCONV", "1") == "1",
               int(_os.environ.get("KWG", "8")))


def _aff(g, b, m, v):
    s = g / np.sqrt(v + EPS)
    return s.astype(np.float32), (b - m * s).astype(np.float32)


def _bf(a):
    return np.asarray(a, np.float32).astype(ml_dtypes.bfloat16)


def _build_nc(cfg=None):
    import concourse.bacc as bacc
    import concourse.mybir as mybir
    import concourse.tile as tile

    cfg = tuple(cfg) if cfg else DEFAULT_CFG
    if len(cfg) == 4:
        cfg = cfg + (8,)
    KRP, KCOLL, KGEMM, KCONV, WG = cfg
    f32, bf16 = mybir.dt.float32, mybir.dt.bfloat16
    nc = bacc.Bacc("TRN2", target_bir_lowering=False, debug=False, num_devices=NC)
    RG = [list(range(NC))]

    def din(name, shape, dt=f32):
        return nc.dram_tensor(name, shape, dt, kind="ExternalInput")

    xin = {}
    xin["x_c"] = din("x_c", [6, NFF])
    xin["x_p"] = din("x_p", [8, NFF])
    xin["x_t"] = din("x_t", [8, NFF])
    xin["x_poi"] = din("x_poi", [12, NFF])
    xin["x_tm"] = din("x_tm", [31, NFF])
    for name, shape in [
        ("wc_T", [6, 64]), ("wp_T", [8, 64]), ("wt_T", [8, 64]),
        ("wtm_T", [31, 28]), ("wtf_T", [28, 1]), ("wpoi_T", [12, 9]),
        ("ones1", [1, 12]), ("W1a_T", [128, 64]), ("W1b_T", [73, 64]),
        ("W3", [9, 64, 56]), ("W4", [9, 72, 64]), ("W2_T", [64, 2]),
        ("ident", [64, 64]),
    ]:
        xin[name] = din(name, shape, bf16)
    for name, shape in [
        ("bcp", [128, 1]), ("btpoi", [73, 1]), ("btm", [28, 1]), ("btf", [1, 1]),
        ("b1", [64, 1]), ("sa", [64, 1]), ("ta", [64, 1]),
        ("s56", [56, 1]), ("bz1", [56, 1]), ("s8", [8, 1]), ("t8", [8, 1]),
        ("b4", [64, 1]), ("b2", [2, 1]), ("bplus", [B, OSH]),
    ]:
        xin[name] = din(name, shape, f32)
    xin["wplus"] = din("wplus", [KCH, 128, OSH], bf16)

    out_ext = nc.dram_tensor("out", [B, 2, HW], f32, kind="ExternalOutput")

    ag_in = [nc.dram_tensor(f"ag_in{i}", [B, OSH], bf16) for i in range(KRP)]
    ag_out = [nc.dram_tensor(f"ag_out{i}", [NC * B, OSH], bf16,
                             addr_space="Shared") for i in range(KRP)]

    Relu = mybir.ActivationFunctionType.Relu
    Tanh = mybir.ActivationFunctionType.Tanh
    TAPS = [(dy, dx) for dy in range(3) for dx in range(3)]

    with tile.TileContext(nc) as tc:
        with (
            tc.tile_pool(name="wsm", bufs=1) as wsm,
            tc.tile_pool(name="act", bufs=1) as act,
            tc.tile_pool(name="wst", bufs=(32 // WG)) as wst,
        ):
            # ---- small weights ----
            wt = {}
            for name in ["wc_T", "wp_T", "wt_T", "wtm_T", "wtf_T", "wpoi_T",
                         "ones1", "W1a_T", "W1b_T", "W2_T", "ident",
                         "bcp", "btpoi", "btm", "btf", "b1", "sa", "ta",
                         "s56", "bz1", "s8", "t8", "b4", "b2", "bplus"]:
                t = wsm.tile(list(xin[name].shape), xin[name].dtype, tag=name)
                nc.sync.dma_start(t[:], xin[name][:])
                wt[name] = t
            w3t = wsm.tile([64, 9 * 56], bf16, tag="W3")
            nc.sync.dma_start(w3t[:].rearrange("c (t o) -> c t o", t=9),
                              xin["W3"].ap().rearrange("t c o -> c t o"))
            w4t = wsm.tile([72, 9 * 64], bf16, tag="W4")
            nc.sync.dma_start(w4t[:].rearrange("c (t o) -> c t o", t=9),
                              xin["W4"].ap().rearrange("t c o -> c t o"))

            # ---- head: x -> cpt [64, NFF] f32 (full batch, chunked) ----
            cpt = act.tile([C, NFF], f32, tag="cpt")
            with (
                tc.tile_pool(name="hx", bufs=2) as hx,
                tc.tile_pool(name="hps", bufs=1, space="PSUM") as hps,
                tc.tile_pool(name="hsb", bufs=2) as hsb,
            ):
                GSZ = 2048
                for grp in range(NFF // GSZ):
                    gsl = slice(grp * GSZ, (grp + 1) * GSZ)
                    xc = hx.tile([6, GSZ], bf16, tag="x_c")
                    xp = hx.tile([8, GSZ], bf16, tag="x_p")
                    xt = hx.tile([8, GSZ], bf16, tag="x_t")
                    xpoi = hx.tile([12, GSZ], bf16, tag="x_poi")
                    xtm = hx.tile([31, GSZ], bf16, tag="x_tm")
                    for t, name in [(xc, "x_c"), (xp, "x_p"), (xt, "x_t"),
                                    (xpoi, "x_poi"), (xtm, "x_tm")]:
                        nc.gpsimd.dma_start(t[:], xin[name][:, gsl])  # casting DMA
                    NQ = 512
                    for qq in range(GSZ // NQ):
                        sl = slice(qq * NQ, (qq + 1) * NQ)
                        osl = slice(grp * GSZ + qq * NQ, grp * GSZ + (qq + 1) * NQ)
                        p_tm = hps.tile([28, NQ], f32, tag="h_tm")
                        nc.tensor.matmul(p_tm[:], wt["wtm_T"][:], xtm[:, sl],
                                         start=True, stop=True)
                        tx = hsb.tile([28, NQ], bf16, tag="h_tx")
                        nc.scalar.activation(tx[:], p_tm[:], Relu, bias=wt["btm"][:])
                        p_tf = hps.tile([1, NQ], f32, tag="h_tf")
                        nc.tensor.matmul(p_tf[:], wt["wtf_T"][:], tx[:],
                                         start=True, stop=True)
                        tx2 = hsb.tile([1, NQ], bf16, tag="h_tx2")
                        nc.scalar.activation(tx2[:], p_tf[:], Relu, bias=wt["btf"][:])
                        p_bc = hps.tile([12, NQ], f32, tag="h_bc")
                        nc.tensor.matmul(p_bc[:], wt["ones1"][:], tx2[:],
                                         start=True, stop=True)
                        xpm = hsb.tile([12, NQ], bf16, tag="h_xpm")
                        nc.vector.tensor_mul(xpm[:], xpoi[:, sl], p_bc[:])

                        p1 = hps.tile([128, NQ], f32, tag="h_p1")
                        nc.tensor.matmul(p1[0:64, :], wt["wc_T"][:], xc[:, sl],
                                         start=True, stop=True)
                        nc.tensor.matmul(p1[64:128, :], wt["wp_T"][:], xp[:, sl],
                                         start=True, stop=True)
                        p2 = hps.tile([73, NQ], f32, tag="h_p2")
                        nc.tensor.matmul(p2[0:64, :], wt["wt_T"][:], xt[:, sl],
                                         start=True, stop=True)
                        nc.tensor.matmul(p2[64:73, :], wt["wpoi_T"][:], xpm[:],
                                         start=True, stop=True)
                        y1 = hsb.tile([128, NQ], bf16, tag="h_y1")
                        nc.scalar.activation(y1[:], p1[:], Relu, bias=wt["bcp"][:])
                        y2 = hsb.tile([73, NQ], bf16, tag="h_y2")
                        nc.scalar.activation(y2[:], p2[:], Relu, bias=wt["btpoi"][:])
                        p_c1 = hps.tile([64, NQ], f32, tag="h_c1")
                        nc.tensor.matmul(p_c1[:], wt["W1a_T"][:], y1[:],
                                         start=True, stop=False)
                        nc.tensor.matmul(p_c1[:], wt["W1b_T"][:], y2[:],
                                         start=False, stop=True)
                        nc.vector.tensor_scalar_add(cpt[:, osl], p_c1[:], wt["b1"][:])

            # ---- iteration-phase pools (reuse head space) ----
            with (
                tc.tile_pool(name="itr", bufs=1) as itr,
                tc.tile_pool(name="zb", bufs=4) as zb,
                tc.tile_pool(name="pg", bufs=1, space="PSUM") as pgp,
                tc.tile_pool(name="pcv", bufs=2, space="PSUM") as pcv,
                tc.tile_pool(name="ptr", bufs=2, space="PSUM") as ptrp,
            ):
                z3pad = itr.tile([72, B * PB], bf16, tag="z3pad")
                nc.vector.memset(z3pad[:], 0.0)
                zt = itr.tile([128, KCH * 32], bf16, tag="zt")
                stage = itr.tile([B, OSH], bf16, tag="stage")

                z3pad_v = z3pad[:].rearrange("c (b y x) -> c b y x", b=B, y=PADR)
                cpt_v = cpt[:].rearrange("c (b y x) -> c b y x", b=B, y=H)

                for it in range(KRP):
                    # z2 = relu(sa * cpt); transpose to zt[p, b*256+sb*64+c]
                    for g in range(8):
                        sg = slice(g * GSZ, (g + 1) * GSZ)
                        z2h = zb.tile([C, GSZ], bf16, tag="z2h")
                        nc.scalar.activation(z2h[:], cpt[:, sg], Relu,
                                             scale=wt["sa"][:])
                        ptb = ptrp.tile([128, 1024], bf16, tag="tr")
                        for t16 in range(16):
                            nc.tensor.transpose(
                                ptb[:, t16 * 64:(t16 + 1) * 64],
                                z2h[:, t16 * 128:(t16 + 1) * 128], wt["ident"][:])
                        nc.vector.tensor_copy(zt[:, g * 1024:(g + 1) * 1024], ptb[:])

                    # GEMM over 256 chunks (never waits on a collective)
                    pg = pgp.tile([B, OSH], f32, tag="g")
                    if not KGEMM:
                        nc.vector.memset(pg[:], 0.0)
                    for jg in range(KCH // WG if KGEMM else 0):
                        wtile = wst.tile([128, WG * OSH], bf16, tag="w")
                        nc.sync.dma_start(
                            wtile[:].rearrange("p (g o) -> p g o", g=WG),
                            xin["wplus"].ap()[jg * WG:jg * WG + WG].rearrange(
                                "g p o -> p g o"))
                        for g in range(WG):
                            j = jg * WG + g
                            c_i, sb_i = j // 4, j % 4
                            lhsT = zt[:, sb_i * 64 + c_i::256]
                            nc.tensor.matmul(
                                pg[:], lhsT, wtile[:, g * OSH:(g + 1) * OSH],
                                start=(j == 0), stop=(j == KCH - 1))
                    nc.vector.tensor_add(stage[:], pg[:], wt["bplus"][:])
                    nc.sync.dma_start(ag_in[it][:], stage[:])
                    if KCOLL:
                        nc.gpsimd.collective_compute(
                            "AllGather", mybir.AluOpType.bypass, replica_groups=RG,
                            ins=[ag_in[it][:].opt()], outs=[ag_out[it][:].opt()])

                    # z1 path - overlaps the AllGather
                    for b in range(B if KCONV else 0):
                        z1p = zb.tile([C, PB], bf16, tag="z1p")
                        nc.vector.memset(z1p[:], 0.0)
                        z1p_v = z1p[:].rearrange("c (y x) -> c y x", y=PADR)
                        nc.scalar.activation(
                            z1p_v[:, 1:33, 1:17], cpt_v[:, b],
                            Relu, scale=wt["sa"][:], bias=wt["ta"][:])
                        pz = pcv.tile([56, HW], f32, tag="cv")
                        for t_i, (dy, dx) in enumerate(TAPS):
                            nc.tensor.matmul(
                                pz[:], w3t[:, t_i * 56:(t_i + 1) * 56],
                                z1p_v[:, dy:dy + 32, dx:dx + 16],
                                start=(t_i == 0), stop=(t_i == 8))
                        nc.scalar.activation(
                            z3pad_v[0:56, b, 1:33, 1:17], pz[:],
                            Relu, scale=wt["s56"][:], bias=wt["bz1"][:])

                    # gathered z2map, per 4-batch group
                    for zg_i in range(8):
                        zmg = zb.tile([8, 4 * HW], bf16, tag="zmg")
                        nc.sync.dma_start(
                            zmg[:].rearrange("j (b s) -> j b s", b=4),
                            ag_out[it].ap().rearrange(
                                "(j b) s -> j b s", j=NC)[:, zg_i * 4:zg_i * 4 + 4])
                        zmg_v = zmg[:].rearrange("j (b y x) -> j b y x", b=4, y=H)
                        for bb in range(4):
                            nc.scalar.activation(
                                z3pad_v[64:72, zg_i * 4 + bb, 1:33, 1:17],
                                zmg_v[:, bb],
                                Relu, scale=wt["s8"][:], bias=wt["t8"][:])

                    # conv2 + residual
                    for b in range(B if KCONV else 0):
                        pc2 = pcv.tile([64, HW], f32, tag="cv")
                        for t_i, (dy, dx) in enumerate(TAPS):
                            nc.tensor.matmul(
                                pc2[:], w4t[:, t_i * 64:(t_i + 1) * 64],
                                z3pad_v[:, b, dy:dy + 32, dx:dx + 16],
                                start=(t_i == 0), stop=(t_i == 8))
                        sl = slice(b * HW, (b + 1) * HW)
                        nc.vector.tensor_add(cpt[:, sl], pc2[:], cpt[:, sl])
                        nc.vector.tensor_scalar_add(cpt[:, sl], cpt[:, sl],
                                                    wt["b4"][:])

                # ---- tail ----
                with tc.tile_pool(name="tps", bufs=2, space="PSUM") as tps:
                    for q in range(NFF // 512):
                        sl = slice(q * 512, (q + 1) * 512)
                        rq = zb.tile([C, 512], bf16, tag="z2h")
                        nc.scalar.activation(rq[:], cpt[:, sl], Relu)
                        po = tps.tile([2, 512], f32, tag="t_o")
                        nc.tensor.matmul(po[:], wt["W2_T"][:], rq[:],
                                         start=True, stop=True)
                        oq = zb.tile([2, 512], f32, tag="oq")
                        nc.scalar.activation(oq[:], po[:], Tanh, bias=wt["b2"][:])
                        nc.sync.dma_start(out_ext.ap()[q], oq[:])

    nc.compile()
    return nc


def _prep_inputs(inputs):
    """Host-side preprocessing -> list of 8 per-core input dicts."""
    ii = {k: np.asarray(v, np.float32) if np.asarray(v).dtype == np.float32
          else np.asarray(v) for k, v in inputs.items()}

    s1, t1 = _aff(ii["bn1_g"], ii["bn1_b"], ii["bn1_m"], ii["bn1_v"])
    sa, ta = _aff(ii["rp_bn1_g"], ii["rp_bn1_b"], ii["rp_bn1_m"], ii["rp_bn1_v"])
    sb_, tb_ = _aff(ii["rp_bn2_g"], ii["rp_bn2_b"], ii["rp_bn2_m"], ii["rp_bn2_v"])
    sc_, tc_ = _aff(ii["bn2_g"], ii["bn2_b"], ii["bn2_m"], ii["bn2_v"])

    conv1_2d = ii["conv1_w"][:, :, 0, 0]
    W1p = conv1_2d * s1[None, :]
    b1p = conv1_2d @ t1 + ii["conv1_b"]
    conv2_2d = ii["conv2_w"][:, :, 0, 0]
    W2p = conv2_2d * sc_[None, :]
    b2p = conv2_2d @ tc_ + ii["conv2_b"]

    xl = ii["x"].transpose(1, 0, 2, 3).reshape(65, NFF)
    base = {
        "x_c": np.ascontiguousarray(xl[0:6]),
        "x_p": np.ascontiguousarray(xl[6:14]),
        "x_t": np.ascontiguousarray(xl[14:22]),
        "x_poi": np.ascontiguousarray(xl[22:34]),
        "x_tm": np.ascontiguousarray(xl[34:65]),
        "wc_T": _bf(ii["convc_w"][:, :, 0, 0].T), "wp_T": _bf(ii["convp_w"][:, :, 0, 0].T),
        "wt_T": _bf(ii["convt_w"][:, :, 0, 0].T), "wtm_T": _bf(ii["tm_w"][:, :, 0, 0].T),
        "wtf_T": _bf(ii["tf_w"][:, :, 0, 0].T), "wpoi_T": _bf(ii["poi_w"][:, :, 0, 0].T),
        "ones1": _bf(np.ones((1, 12))), "W1a_T": _bf(W1p[:, :128].T),
        "W1b_T": _bf(W1p[:, 128:].T),
        "W3": _bf(np.stack([ii["rp_conv1_w"][:, :, dy, dx].T
                            for dy in range(3) for dx in range(3)])),
        "W4": _bf(np.stack([np.concatenate([
                            ii["rp_conv2_w"][:, :56, dy, dx].T,
                            np.zeros((8, 64), np.float32),
                            ii["rp_conv2_w"][:, 56:, dy, dx].T])
                            for dy in range(3) for dx in range(3)])),
        "W2_T": _bf(W2p.T), "ident": _bf(np.eye(64)),
        "bcp": np.concatenate([ii["convc_b"], ii["convp_b"]])[:, None].astype(np.float32),
        "btpoi": np.concatenate([ii["convt_b"], ii["poi_b"]])[:, None].astype(np.float32),
        "btm": ii["tm_b"][:, None], "btf": ii["tf_b"][:, None],
        "b1": b1p[:, None].astype(np.float32), "sa": sa[:, None], "ta": ta[:, None],
        "s56": sb_[:56, None],
        "bz1": (sb_[:56] * ii["rp_conv1_b"] + tb_[:56])[:, None].astype(np.float32),
        "s8": sb_[56:, None], "t8": tb_[56:, None],
        "b4": ii["rp_conv2_b"][:, None], "b2": b2p[:, None].astype(np.float32),
    }

    plus_wf = ii["plus_w"].reshape(8 * HW, C * HW)
    ta_flat = np.repeat(ta, HW)
    in_maps = []
    for c in range(NC):
        m = dict(base)
        Wsh = plus_wf[c * OSH:(c + 1) * OSH]
        bias_eff = (ii["plus_b"][c * OSH:(c + 1) * OSH] + Wsh @ ta_flat).astype(np.float32)
        m["bplus"] = np.broadcast_to(bias_eff, (B, OSH)).copy()
        m["wplus"] = np.ascontiguousarray(_bf(Wsh.T).reshape(KCH, 128, OSH))
        in_maps.append(m)
    return in_maps


def _build_sharded(nc):
    import jax
    import numpy as _np
    from jax.sharding import Mesh, PartitionSpec
    from jax.experimental.shard_map import shard_map
    import concourse.mybir as mybir
    from concourse.bass2jax import (_bass_exec_p, install_neuronx_cc_hook,
                                    partition_id_tensor)

    install_neuronx_cc_hook()
    partition_name = nc.partition_id_tensor.name if nc.partition_id_tensor else None
    in_names, out_names, out_avals, zero_outs = [], [], [], []
    for alloc in nc.m.functions[0].allocations:
        if not isinstance(alloc, mybir.MemoryLocationSet):
            continue
        name = alloc.memorylocations[0].name
        if alloc.kind == "ExternalInput":
            if name != partition_name:
                in_names.append(name)
        elif alloc.kind == "ExternalOutput":
            shape = tuple(alloc.tensor_shape)
            dtype = mybir.dt.np(alloc.dtype)
            out_avals.append(jax.core.ShapedArray(shape, dtype))
            out_names.append(name)
            zero_outs.append(_np.zeros(shape, dtype))
    n_params = len(in_names)
    n_outs = len(out_avals)
    all_in_names = list(in_names) + list(out_names)
    if partition_name is not None:
        all_in_names.append(partition_name)
    donate = tuple(range(n_params, n_params + n_outs))

    def _body(*args):
        operands = list(args)
        if partition_name is not None:
            operands.append(partition_id_tensor())
        outs = _bass_exec_p.bind(
            *operands, out_avals=tuple(out_avals), in_names=tuple(all_in_names),
            out_names=tuple(out_names), lowering_input_output_aliases=(),
            sim_require_finite=True, sim_require_nnan=True, nc=nc)
        return tuple(outs)

    devices = jax.devices()[:NC]
    mesh = Mesh(_np.asarray(devices), ("core",))
    in_specs = (PartitionSpec("core"),) * (n_params + n_outs)
    out_specs = (PartitionSpec("core"),) * n_outs
    fn = jax.jit(
        shard_map(_body, mesh=mesh, in_specs=in_specs, out_specs=out_specs,
                  check_rep=False),
        donate_argnums=donate, keep_unused=True)
    return dict(fn=fn, in_names=in_names, out_names=out_names,
                out_avals=out_avals, zero_outs=zero_outs)


def get_compiled(cfg=None):
    key = tuple(cfg) if cfg else DEFAULT_CFG
    if len(key) == 4:
        key = key + (8,)
    if key not in _HANDLE:
        nc = _build_nc(key)
        _HANDLE[key] = _build_sharded(nc)
    return _HANDLE[key]


def stage_inputs(in_maps, cfg=None):
    """device_put concatenated inputs once; returns device arrays."""
    import jax
    import numpy as _np
    from jax.sharding import Mesh, NamedSharding, PartitionSpec
    h = get_compiled(cfg)
    mesh = Mesh(_np.asarray(jax.devices()[:NC]), ("core",))
    sh = NamedSharding(mesh, PartitionSpec("core"))
    concat_in = [_np.concatenate([_np.asarray(in_maps[c][name]) for c in range(NC)],
                                 axis=0) for name in h["in_names"]]
    return [jax.device_put(a, sh) for a in concat_in]


def run_staged(staged, cfg=None):
    import jax
    import numpy as _np
    h = get_compiled(cfg)
    concat_zeros = [_np.zeros((NC * z.shape[0], *z.shape[1:]), z.dtype)
                    for z in h["zero_outs"]]
    out_arrs = h["fn"](*staged, *concat_zeros)
    jax.block_until_ready(out_arrs)
    return [
        {name: _np.asarray(out_arrs[i]).reshape(NC, *h["out_avals"][i].shape)[c]
         for i, name in enumerate(h["out_names"])}
        for c in range(NC)
    ]


def run_spmd(in_maps, cfg=None):
    import jax
    import numpy as _np
    h = get_compiled(cfg)
    concat_in = [_np.concatenate([_np.asarray(in_maps[c][name]) for c in range(NC)],
                                 axis=0) for name in h["in_names"]]
    concat_zeros = [_np.zeros((NC * z.shape[0], *z.shape[1:]), z.dtype)
                    for z in h["zero_outs"]]
    out_arrs = h["fn"](*concat_in, *concat_zeros)
    jax.block_until_ready(out_arrs)
    return [
        {name: _np.asarray(out_arrs[i]).reshape(NC, *h["out_avals"][i].shape)[c]
         for i, name in enumerate(h["out_names"])}
        for c in range(NC)
    ]


def kernel(**inputs):
    in_maps = _prep_inputs(inputs)
    results = run_spmd(in_maps)
    return results[0]["out"].reshape(B, 2, H, W).astype(np.float32)

